# revision 14
# baseline (speedup 1.0000x reference)
"""Causal single-head attention (B=16, T=2048, C=HEAD=384) on 8 trn2 cores.

Sharding: data-parallel over batch. Each core gets 2 batch elements and
runs the identical Bass program; results are concatenated on the host.

Math trick: scores = q @ k^T = x @ (Wq Wk^T) @ x^T. The host precomputes
TT = Wk Wq^T (weight-only prep), so per batch the kernel computes a
single projection kAT = TT^T-contracted projection of x^T (instead of
both q and k); the scores matmul streams x^T directly:
    scoresT[s, t] = sum_a kAT[a, s] * xT[a, t].

DMA-descriptor layouts (the DMA rings are descriptor-rate bound, so every
HBM-side run must be long and contiguous):
  * x is loaded interleaved: t = 512a + 4p + n (a = query group, p =
    partition, n in [0,4)), one 6KB contiguous run per (partition, a).
    The PE transposes (needed anyway for x^T) absorb the layout; the
    psum->xT copies write contiguous and read psum strided (cheap).
  * TT/Wv are loaded with the contraction axis permuted: partition q
    holds rows c = 3q+j (4.6KB runs). A contraction axis only needs
    consistent ordering on both matmul operands, so x's c-axis is
    deinterleaved to the same sigma order during the bf16 cast, and TT's
    column axis is sigma-permuted on the host.
  * Output DMAs issue from the scalar engine's HW ring so their waits
    don't block next-batch input DMAs on the sync ring.

Per-core program (per batch element):
  1. DMA x in 4 group-chunks, cast bf16 (deinterleaving c), PE-transpose
     into per-(sigma-chunk, group) xT tiles [128, 512].
  2. kAT = TT-contraction @ x^T; v = x @ Wv ([T, HEAD+1], last col = 1).
  3. Per 512-wide query group g, per causal key block jb: scoresT in
     PSUM fp32 (diagonal blocks narrowed + additive causal mask), evict
     with ACT exp(scale * .) -> bf16.
  4. PV: out = sum_jb weiT^T @ v_ext; the ones column gives the softmax
     denominator in out[:, C]; multiply by its reciprocal, DMA out.

No max-subtraction in softmax: scores*scale are ~N(0,1) for these inputs
so exp cannot overflow fp32; mathematically identical to the reference.
"""

import os
import sys

import numpy as np

for _p in ("/opt/trn_rl_repo",):
    if os.path.isdir(_p) and _p not in sys.path:
        sys.path.append(_p)

B, T, C = 16, 2048, 384
N_CORES = 8
BPC = B // N_CORES  # batch elements per core
P = 128
NCC = C // P  # 3 contraction chunks over C (and over HEAD, since HEAD == C)
GW = 512  # query-group width
NI = GW // P  # 4: interleave factor (inner rows per partition per group)
SCALE = float(C) ** -0.5
MASK_BIG = -1e9

# Compute dtype for matmul operands: "bf16" (fastest), "f32r", or "f32".
CDT_NAME = os.environ.get("ATTN_CDT", "bf16")

_cache = {}


def _build(bpc, t, c):
    import concourse.bass as bass  # noqa: F401
    import concourse.mybir as mybir
    from concourse import bacc
    from concourse.masks import make_identity
    from concourse.tile import TileContext

    f32 = mybir.dt.float32
    nt = t // P  # 16 t-blocks
    ng = t // GW  # 4 query groups (= x DMA chunks per batch)

    if CDT_NAME == "bf16":
        cdt = mybir.dt.bfloat16
        mm_cast = None
    elif CDT_NAME == "f32r":
        cdt = f32
        mm_cast = mybir.dt.float32r
    else:
        cdt = f32
        mm_cast = None

    def mm(ap):
        return ap.bitcast(mm_cast) if mm_cast is not None else ap

    nc = bacc.Bacc("TRN2", target_bir_lowering=False)

    x_d = nc.declare_dram_parameter("x", [bpc, t, c], f32, isOutput=False)
    tt_d = nc.declare_dram_parameter("tt", [c, c], f32, isOutput=False)
    wv_d = nc.declare_dram_parameter("wv", [c, c], f32, isOutput=False)
    y_d = nc.declare_dram_parameter("y", [bpc, t, c], f32, isOutput=True)

    small = cdt != f32
    cast_x = cdt != f32

    with TileContext(nc) as tc:
        with (
            tc.tile_pool(name="singles", bufs=1) as singles,
            tc.tile_pool(name="wstage", bufs=1) as wstage,
            tc.tile_pool(name="xf", bufs=3) as xf_pool,
            tc.tile_pool(name="xb", bufs=3) as xb_pool,
            tc.tile_pool(name="xT", bufs=2 if small else 1) as xT_pool,
            tc.tile_pool(name="kAT", bufs=2 if small else 1) as kAT_pool,
            tc.tile_pool(name="v", bufs=nt + 4 if small else nt + 1) as v_pool,
            tc.tile_pool(name="wT", bufs=nt + 4 if small else nt + 1) as wT_pool,
            tc.tile_pool(name="outp", bufs=4) as out_pool,
            tc.tile_pool(name="ps_t", bufs=2, space="PSUM") as ps_t,
            tc.tile_pool(name="ps_kv", bufs=2, space="PSUM") as ps_kv,
            tc.tile_pool(name="ps_sc", bufs=2, space="PSUM") as ps_sc,
            tc.tile_pool(name="ps_pv", bufs=2, space="PSUM") as ps_pv,
        ):
            ident = singles.tile([P, P], cdt)
            make_identity(nc, ident)

            # HAM warm-up: dummy matmuls with no input deps run during the
            # initial DMA wait, so the PE clock-gate is at 8/8 when real
            # work arrives. ~50 x N=128 covers the ~3.4us SHORT window.
            wu = singles.tile([P, P], cdt, name="wu", tag="wu")
            nc.vector.memset(wu, 0.0)
            for _ in range(48):
                pswu = ps_t.tile([P, P], f32, name="pswu", tag="pst")
                nc.tensor.matmul(pswu, mm(wu), mm(wu), start=True, stop=True)

            # mask[s, 3P + u] = 0 if u >= s else MASK_BIG; the slice
            # mask[:, 3P : 3P + N] masks every (narrowed) diagonal block.
            mw = GW + 3 * P
            mask = singles.tile([P, mw], f32)
            nc.gpsimd.memset(mask, 0.0)
            nc.gpsimd.affine_select(
                out=mask,
                in_=mask,
                compare_op=mybir.AluOpType.is_ge,
                fill=MASK_BIG,
                base=-3 * P,
                pattern=[[1, mw]],
                channel_multiplier=-1,
            )

            # ---- batch-0 x DMAs first so the sync ring starts on them ----
            xT_all = []  # per batch: xT[j][a] tiles
            xq_all = []  # per batch: list of (xf, xb) per chunk
            for b in range(bpc):
                xT_all.append(
                    [
                        [
                            xT_pool.tile(
                                [P, GW], cdt, name=f"xT{j}_{a}", tag=f"xT{j}_{a}"
                            )
                            for a in range(ng)
                        ]
                        for j in range(NCC)
                    ]
                )

            def load_x_chunk(b, a):
                # chunk a: rows t = 512a + 4p + n; per-partition 6KB run
                xv = x_d[b].rearrange("(a p n) c -> p a n c", a=ng, n=NI)
                xf = xf_pool.tile([P, NI, c], f32, name="xf", tag="xf")
                nc.sync.dma_start(out=xf, in_=xv[:, a, :, :])
                if cast_x:
                    # deinterleave c to sigma order during the cast:
                    # xb[:, n, j, qq] = x[.., c=3qq+j]
                    xb = xb_pool.tile([P, NI, NCC, P], cdt, name="xb", tag="xb")
                    nc.vector.tensor_copy(
                        xb, xf.rearrange("p n (qq j) -> p n j qq", j=NCC)
                    )
                    return xb
                return xf.rearrange("p n (qq j) -> p n j qq", j=NCC)

            def transpose_x_chunk(b, a, xb):
                for j in range(NCC):
                    pst = ps_t.tile([P, NI, P], cdt, name="pst", tag="pst")
                    for n in range(NI):
                        nc.tensor.transpose(pst[:, n, :], xb[:, n, j, :], ident)
                    # dest position 4*p + n == t_local; contiguous writes,
                    # strided psum reads
                    nc.vector.tensor_copy(
                        xT_all[b][j][a].rearrange("q (p4 n4) -> q p4 n4", n4=NI),
                        pst.rearrange("q n p -> q p n"),
                    )

            # batch-0 chunk 0 first on the sync ring, split into two
            # half-chunks so PE starts as early as possible
            xv0 = x_d[0].rearrange("(a p n) c -> p a n c", a=ng, n=NI)
            for h in range(2):
                xfh = xf_pool.tile([P, NI // 2, c], f32, name="xfh", tag="xf")
                nc.sync.dma_start(
                    out=xfh, in_=xv0[:, 0, h * (NI // 2) : (h + 1) * (NI // 2), :]
                )
                if cast_x:
                    xbh = xb_pool.tile(
                        [P, NI // 2, NCC, P], cdt, name="xbh", tag="xb"
                    )
                    nc.vector.tensor_copy(
                        xbh, xfh.rearrange("p n (qq j) -> p n j qq", j=NCC)
                    )
                else:
                    xbh = xfh.rearrange("p n (qq j) -> p n j qq", j=NCC)
                for j in range(NCC):
                    psh = ps_t.tile([P, NI // 2, P], cdt, name="psh", tag="pst")
                    for n in range(NI // 2):
                        nc.tensor.transpose(psh[:, n, :], xbh[:, n, j, :], ident)
                    # dest t_local = 4*p + (2h + n): stride-4 pairs
                    nc.vector.tensor_copy(
                        xT_all[0][j][0].rearrange("q (p4 n4) -> q p4 n4", n4=NI)[
                            :, :, h * (NI // 2) : (h + 1) * (NI // 2)
                        ],
                        psh.rearrange("q n p -> q p n"),
                    )

            # ---- weights (rows c = 3q+j per partition; 4.6KB runs) ----
            w_sb = []
            for name, wd in (("tt", tt_d), ("wv", wv_d)):
                stage = wstage.tile(
                    [P, NCC, c], f32, name=f"wst_{name}", tag=f"wst_{name}"
                )
                nc.sync.dma_start(
                    out=stage, in_=wd[:].rearrange("(q j) h -> q j h", j=NCC)
                )
                if cdt == f32:
                    w_sb.append(stage)
                else:
                    wb = singles.tile(
                        [P, NCC, c], cdt, name=f"{name}b", tag=f"{name}b"
                    )
                    nc.vector.tensor_copy(wb, stage)
                    w_sb.append(wb)
            TT, WV = w_sb

            for b in range(bpc):
                xT = xT_all[b]
                kAT = [
                    [
                        kAT_pool.tile(
                            [P, GW], cdt, name=f"kAT{ca}_{a}", tag=f"kAT{ca}_{a}"
                        )
                        for a in range(ng)
                    ]
                    for ca in range(NCC)
                ]
                v_t = []

                # chunk-major: transpose(a) -> kAT(a) -> v(a) -> attention(a)
                # keeps PE dense from the first chunk onward.
                for g in range(ng):
                    if b > 0 or g > 0:
                        xbg = load_x_chunk(b, g)
                        transpose_x_chunk(b, g, xbg)

                    # kAT for this chunk
                    for ca in range(NCC):
                        ps = ps_kv.tile([P, GW], f32, name="pskv", tag="kv")
                        for cc in range(NCC):
                            nc.tensor.matmul(
                                ps,
                                mm(TT[:, cc, ca * P : (ca + 1) * P]),
                                mm(xT[cc][g]),
                                start=(cc == 0),
                                stop=(cc == NCC - 1),
                            )
                        nc.vector.tensor_copy(kAT[ca][g], ps)

                    # v for this chunk's 4 t-blocks
                    for n in range(NI * g, NI * g + NI):
                        vt = v_pool.tile([P, c + 1], cdt, name="vt", tag="v")
                        ps = ps_kv.tile([P, GW], f32, name="pskv", tag="kv")
                        for cc in range(NCC):
                            nc.tensor.matmul(
                                ps[:, :c],
                                mm(
                                    xT[cc][n // NI][
                                        :, (n % NI) * P : (n % NI + 1) * P
                                    ]
                                ),
                                mm(WV[:, cc, :]),
                                start=(cc == 0),
                                stop=(cc == NCC - 1),
                            )
                        nc.vector.tensor_copy(vt[:, :c], ps[:, :c])
                        nc.vector.memset(vt[:, c : c + 1], 1.0)
                        v_t.append(vt)

                    # ---- attention for query group g ----
                    nblk = NI * g + NI  # causal: s-blocks 0 .. 4g+3
                    wT = []  # (tile, first-valid t_local) per jb
                    for jb in range(nblk):
                        dv = jb - NI * g  # >= 0: diagonal block, narrowed
                        off = max(dv, 0) * P
                        n_free = GW - off
                        ps = ps_sc.tile([P, GW], f32, name="pssc", tag="sc")
                        for cc in range(NCC):
                            nc.tensor.matmul(
                                ps[:, :n_free],
                                mm(
                                    kAT[cc][jb // NI][
                                        :, (jb % NI) * P : (jb % NI + 1) * P
                                    ]
                                ),
                                mm(xT[cc][g][:, off:]),
                                start=(cc == 0),
                                stop=(cc == NCC - 1),
                            )
                        if dv >= 0:
                            nc.vector.tensor_add(
                                ps[:, :n_free],
                                ps[:, :n_free],
                                mask[:, 3 * P : 3 * P + n_free],
                            )
                        wt = wT_pool.tile([P, GW], cdt, name="wTt", tag="wT")
                        nc.scalar.activation(
                            out=wt[:, :n_free],
                            in_=ps[:, :n_free],
                            func=mybir.ActivationFunctionType.Exp,
                            scale=SCALE,
                        )
                        wT.append((wt, off))

                    for il in range(NI):
                        ti = NI * g + il
                        ps_o = ps_pv.tile([P, c + 1], f32, name="psmo", tag="pv")
                        for jb in range(ti + 1):
                            wt, off = wT[jb]
                            lo = il * P - off
                            nc.tensor.matmul(
                                ps_o,
                                mm(wt[:, lo : lo + P]),
                                mm(v_t[jb][:]),
                                start=(jb == 0),
                                stop=(jb == ti),
                            )
                        recip = out_pool.tile([P, 1], f32, name="recip", tag="recip")
                        nc.vector.reciprocal(recip, ps_o[:, c : c + 1])
                        ob = out_pool.tile([P, c], f32, name="ob", tag="ob")
                        nc.vector.tensor_scalar_mul(ob, ps_o[:, :c], recip)
                        # output on the scalar HW ring (keeps sync ring free
                        # for next-batch input prefetch)
                        nc.scalar.dma_start(
                            out=y_d[b, ti * P : (ti + 1) * P, :], in_=ob
                        )

    nc.compile()
    return nc


def _get_nc(bpc, t, c):
    key = (bpc, t, c, CDT_NAME)
    if key not in _cache:
        _cache[key] = _build(bpc, t, c)
    return _cache[key]


def run(x, Wq, Wk, Wv, trace=False):
    """Run on hardware; returns (y, BassKernelResults)."""
    from concourse.bass_utils import run_bass_kernel_spmd

    x = np.ascontiguousarray(np.asarray(x, dtype=np.float32))
    Wq = np.asarray(Wq, dtype=np.float32)
    Wk = np.asarray(Wk, dtype=np.float32)
    Wv = np.ascontiguousarray(np.asarray(Wv, dtype=np.float32))
    b, t, c = x.shape
    assert b % N_CORES == 0
    bpc = b // N_CORES

    # Host weight prep: TT = Wk Wq^T with columns in sigma order
    # (position ca*128+qa holds a = 3*qa+ca, matching the device layout).
    tt = (Wk.astype(np.float64) @ Wq.astype(np.float64).T).astype(np.float32)
    perm = np.concatenate([3 * np.arange(P) + ca for ca in range(NCC)])
    tt = np.ascontiguousarray(tt[:, perm])

    nc = _get_nc(bpc, t, c)
    core_ids = list(range(N_CORES))
    in_maps = [
        {"x": x[i * bpc : (i + 1) * bpc], "tt": tt, "wv": Wv}
        for i in core_ids
    ]
    res = run_bass_kernel_spmd(nc, in_maps, core_ids, trace=trace)
    y = np.concatenate([res.results[i]["y"] for i in core_ids], axis=0)
    return y, res


def kernel(x, Wq, Wk, Wv):
    y, _ = run(x, Wq, Wk, Wv, trace=False)
    return y


# revision 15
# speedup vs baseline: 1.0065x; 1.0065x over previous
"""Causal single-head attention (B=16, T=2048, C=HEAD=384) on 8 trn2 cores.

Sharding: data-parallel over batch. Each core gets 2 batch elements and
runs the identical Bass program; results are concatenated on the host.

Math trick: scores = q @ k^T = x @ (Wq Wk^T) @ x^T. The host precomputes
TT = Wk Wq^T (weight-only prep), so per batch the kernel computes a
single projection kAT = TT^T-contracted projection of x^T (instead of
both q and k); the scores matmul streams x^T directly:
    scoresT[s, t] = sum_a kAT[a, s] * xT[a, t].

DMA-descriptor layouts (the DMA rings are descriptor-rate bound, so every
HBM-side run must be long and contiguous):
  * x is loaded interleaved: t = 512a + 4p + n (a = query group, p =
    partition, n in [0,4)), one 6KB contiguous run per (partition, a).
    The PE transposes (needed anyway for x^T) absorb the layout; the
    psum->xT copies write contiguous and read psum strided (cheap).
  * TT/Wv are loaded with the contraction axis permuted: partition q
    holds rows c = 3q+j (4.6KB runs). A contraction axis only needs
    consistent ordering on both matmul operands, so x's c-axis is
    deinterleaved to the same sigma order during the bf16 cast, and TT's
    column axis is sigma-permuted on the host.
  * Output DMAs issue from the scalar engine's HW ring so their waits
    don't block next-batch input DMAs on the sync ring.

Per-core program (per batch element):
  1. DMA x in 4 group-chunks, cast bf16 (deinterleaving c), PE-transpose
     into per-(sigma-chunk, group) xT tiles [128, 512].
  2. kAT = TT-contraction @ x^T; v = x @ Wv ([T, HEAD+1], last col = 1).
  3. Per 512-wide query group g, per causal key block jb: scoresT in
     PSUM fp32 (diagonal blocks narrowed + additive causal mask), evict
     with ACT exp(scale * .) -> bf16.
  4. PV: out = sum_jb weiT^T @ v_ext; the ones column gives the softmax
     denominator in out[:, C]; multiply by its reciprocal, DMA out.

No max-subtraction in softmax: scores*scale are ~N(0,1) for these inputs
so exp cannot overflow fp32; mathematically identical to the reference.
"""

import os
import sys

import numpy as np

for _p in ("/opt/trn_rl_repo",):
    if os.path.isdir(_p) and _p not in sys.path:
        sys.path.append(_p)

B, T, C = 16, 2048, 384
N_CORES = 8
BPC = B // N_CORES  # batch elements per core
P = 128
NCC = C // P  # 3 contraction chunks over C (and over HEAD, since HEAD == C)
GW = 512  # query-group width
NI = GW // P  # 4: interleave factor (inner rows per partition per group)
SCALE = float(C) ** -0.5
MASK_BIG = -1e9

# Compute dtype for matmul operands: "bf16" (fastest), "f32r", or "f32".
CDT_NAME = os.environ.get("ATTN_CDT", "bf16")

_cache = {}


def _build(bpc, t, c):
    import concourse.bass as bass  # noqa: F401
    import concourse.mybir as mybir
    from concourse import bacc
    from concourse.masks import make_identity
    from concourse.tile import TileContext

    f32 = mybir.dt.float32
    nt = t // P  # 16 t-blocks
    ng = t // GW  # 4 query groups (= x DMA chunks per batch)

    if CDT_NAME == "bf16":
        cdt = mybir.dt.bfloat16
        mm_cast = None
    elif CDT_NAME == "f32r":
        cdt = f32
        mm_cast = mybir.dt.float32r
    else:
        cdt = f32
        mm_cast = None

    def mm(ap):
        return ap.bitcast(mm_cast) if mm_cast is not None else ap

    nc = bacc.Bacc("TRN2", target_bir_lowering=False)

    x_d = nc.declare_dram_parameter("x", [bpc, t, c], f32, isOutput=False)
    tt_d = nc.declare_dram_parameter("tt", [c, c], f32, isOutput=False)
    wv_d = nc.declare_dram_parameter("wv", [c, c], f32, isOutput=False)
    y_d = nc.declare_dram_parameter("y", [bpc, t, c], f32, isOutput=True)

    small = cdt != f32
    cast_x = cdt != f32

    with TileContext(nc) as tc:
        with (
            tc.tile_pool(name="singles", bufs=1) as singles,
            tc.tile_pool(name="wstage", bufs=1) as wstage,
            tc.tile_pool(name="xf", bufs=3) as xf_pool,
            tc.tile_pool(name="xb", bufs=3) as xb_pool,
            tc.tile_pool(name="xT", bufs=2 if small else 1) as xT_pool,
            tc.tile_pool(name="kAT", bufs=2 if small else 1) as kAT_pool,
            tc.tile_pool(name="v", bufs=nt + 4 if small else nt + 1) as v_pool,
            tc.tile_pool(name="wT", bufs=nt + 4 if small else nt + 1) as wT_pool,
            tc.tile_pool(name="outp", bufs=4) as out_pool,
            tc.tile_pool(name="ps_t", bufs=2, space="PSUM") as ps_t,
            tc.tile_pool(name="ps_kv", bufs=2, space="PSUM") as ps_kv,
            tc.tile_pool(name="ps_sc", bufs=2, space="PSUM") as ps_sc,
            tc.tile_pool(name="ps_pv", bufs=2, space="PSUM") as ps_pv,
        ):
            ident = singles.tile([P, P], cdt)
            make_identity(nc, ident)

            # HAM warm-up: dummy matmuls with no input deps run during the
            # initial DMA wait, so the PE clock-gate is at 8/8 when real
            # work arrives. ~50 x N=128 covers the ~3.4us SHORT window.
            wu = singles.tile([P, GW], cdt, name="wu", tag="wu")
            nc.vector.memset(wu, 0.0)
            for _ in range(18):
                pswu = ps_t.tile([P, GW], f32, name="pswu", tag="pst")
                nc.tensor.matmul(
                    pswu, mm(wu[:, :P]), mm(wu), start=True, stop=True
                )

            # mask[s, 3P + u] = 0 if u >= s else MASK_BIG; the slice
            # mask[:, 3P : 3P + N] masks every (narrowed) diagonal block.
            mw = GW + 3 * P
            mask = singles.tile([P, mw], f32)
            nc.gpsimd.memset(mask, 0.0)
            nc.gpsimd.affine_select(
                out=mask,
                in_=mask,
                compare_op=mybir.AluOpType.is_ge,
                fill=MASK_BIG,
                base=-3 * P,
                pattern=[[1, mw]],
                channel_multiplier=-1,
            )

            # ---- batch-0 x DMAs first so the sync ring starts on them ----
            xT_all = []  # per batch: xT[j][a] tiles
            xq_all = []  # per batch: list of (xf, xb) per chunk
            for b in range(bpc):
                xT_all.append(
                    [
                        [
                            xT_pool.tile(
                                [P, GW], cdt, name=f"xT{j}_{a}", tag=f"xT{j}_{a}"
                            )
                            for a in range(ng)
                        ]
                        for j in range(NCC)
                    ]
                )

            def load_x_chunk(b, a):
                # chunk a: rows t = 512a + 4p + n; per-partition 6KB run
                xv = x_d[b].rearrange("(a p n) c -> p a n c", a=ng, n=NI)
                xf = xf_pool.tile([P, NI, c], f32, name="xf", tag="xf")
                nc.sync.dma_start(out=xf, in_=xv[:, a, :, :])
                if cast_x:
                    # deinterleave c to sigma order during the cast:
                    # xb[:, n, j, qq] = x[.., c=3qq+j]
                    xb = xb_pool.tile([P, NI, NCC, P], cdt, name="xb", tag="xb")
                    nc.vector.tensor_copy(
                        xb, xf.rearrange("p n (qq j) -> p n j qq", j=NCC)
                    )
                    return xb
                return xf.rearrange("p n (qq j) -> p n j qq", j=NCC)

            def transpose_x_chunk(b, a, xb):
                for j in range(NCC):
                    pst = ps_t.tile([P, NI, P], cdt, name="pst", tag="pst")
                    for n in range(NI):
                        nc.tensor.transpose(pst[:, n, :], xb[:, n, j, :], ident)
                    # dest position 4*p + n == t_local; contiguous writes,
                    # strided psum reads
                    nc.vector.tensor_copy(
                        xT_all[b][j][a].rearrange("q (p4 n4) -> q p4 n4", n4=NI),
                        pst.rearrange("q n p -> q p n"),
                    )

            # batch-0 chunk 0 first on the sync ring, split into two
            # half-chunks so PE starts as early as possible
            xv0 = x_d[0].rearrange("(a p n) c -> p a n c", a=ng, n=NI)
            for h in range(2):
                xfh = xf_pool.tile([P, NI // 2, c], f32, name="xfh", tag="xf")
                nc.sync.dma_start(
                    out=xfh, in_=xv0[:, 0, h * (NI // 2) : (h + 1) * (NI // 2), :]
                )
                if cast_x:
                    xbh = xb_pool.tile(
                        [P, NI // 2, NCC, P], cdt, name="xbh", tag="xb"
                    )
                    nc.vector.tensor_copy(
                        xbh, xfh.rearrange("p n (qq j) -> p n j qq", j=NCC)
                    )
                else:
                    xbh = xfh.rearrange("p n (qq j) -> p n j qq", j=NCC)
                for j in range(NCC):
                    psh = ps_t.tile([P, NI // 2, P], cdt, name="psh", tag="pst")
                    for n in range(NI // 2):
                        nc.tensor.transpose(psh[:, n, :], xbh[:, n, j, :], ident)
                    # dest t_local = 4*p + (2h + n): stride-4 pairs
                    nc.vector.tensor_copy(
                        xT_all[0][j][0].rearrange("q (p4 n4) -> q p4 n4", n4=NI)[
                            :, :, h * (NI // 2) : (h + 1) * (NI // 2)
                        ],
                        psh.rearrange("q n p -> q p n"),
                    )

            # ---- weights (rows c = 3q+j per partition; 4.6KB runs) ----
            w_sb = []
            for name, wd in (("tt", tt_d), ("wv", wv_d)):
                stage = wstage.tile(
                    [P, NCC, c], f32, name=f"wst_{name}", tag=f"wst_{name}"
                )
                nc.sync.dma_start(
                    out=stage, in_=wd[:].rearrange("(q j) h -> q j h", j=NCC)
                )
                if cdt == f32:
                    w_sb.append(stage)
                else:
                    wb = singles.tile(
                        [P, NCC, c], cdt, name=f"{name}b", tag=f"{name}b"
                    )
                    nc.vector.tensor_copy(wb, stage)
                    w_sb.append(wb)
            TT, WV = w_sb

            for b in range(bpc):
                xT = xT_all[b]
                kAT = [
                    [
                        kAT_pool.tile(
                            [P, GW], cdt, name=f"kAT{ca}_{a}", tag=f"kAT{ca}_{a}"
                        )
                        for a in range(ng)
                    ]
                    for ca in range(NCC)
                ]
                v_t = []

                # chunk-major: transpose(a) -> kAT(a) -> v(a) -> attention(a)
                # keeps PE dense from the first chunk onward.
                for g in range(ng):
                    if b > 0 or g > 0:
                        xbg = load_x_chunk(b, g)
                        transpose_x_chunk(b, g, xbg)

                    # kAT for this chunk
                    for ca in range(NCC):
                        ps = ps_kv.tile([P, GW], f32, name="pskv", tag="kv")
                        for cc in range(NCC):
                            nc.tensor.matmul(
                                ps,
                                mm(TT[:, cc, ca * P : (ca + 1) * P]),
                                mm(xT[cc][g]),
                                start=(cc == 0),
                                stop=(cc == NCC - 1),
                            )
                        nc.vector.tensor_copy(kAT[ca][g], ps)

                    # v for this chunk's 4 t-blocks
                    for n in range(NI * g, NI * g + NI):
                        vt = v_pool.tile([P, c + 1], cdt, name="vt", tag="v")
                        ps = ps_kv.tile([P, GW], f32, name="pskv", tag="kv")
                        for cc in range(NCC):
                            nc.tensor.matmul(
                                ps[:, :c],
                                mm(
                                    xT[cc][n // NI][
                                        :, (n % NI) * P : (n % NI + 1) * P
                                    ]
                                ),
                                mm(WV[:, cc, :]),
                                start=(cc == 0),
                                stop=(cc == NCC - 1),
                            )
                        nc.vector.tensor_copy(vt[:, :c], ps[:, :c])
                        nc.vector.memset(vt[:, c : c + 1], 1.0)
                        v_t.append(vt)

                    # ---- attention for query group g ----
                    nblk = NI * g + NI  # causal: s-blocks 0 .. 4g+3
                    wT = []  # (tile, first-valid t_local) per jb
                    for jb in range(nblk):
                        dv = jb - NI * g  # >= 0: diagonal block, narrowed
                        off = max(dv, 0) * P
                        n_free = GW - off
                        ps = ps_sc.tile([P, GW], f32, name="pssc", tag="sc")
                        for cc in range(NCC):
                            nc.tensor.matmul(
                                ps[:, :n_free],
                                mm(
                                    kAT[cc][jb // NI][
                                        :, (jb % NI) * P : (jb % NI + 1) * P
                                    ]
                                ),
                                mm(xT[cc][g][:, off:]),
                                start=(cc == 0),
                                stop=(cc == NCC - 1),
                            )
                        if dv >= 0:
                            nc.vector.tensor_add(
                                ps[:, :n_free],
                                ps[:, :n_free],
                                mask[:, 3 * P : 3 * P + n_free],
                            )
                        wt = wT_pool.tile([P, GW], cdt, name="wTt", tag="wT")
                        nc.scalar.activation(
                            out=wt[:, :n_free],
                            in_=ps[:, :n_free],
                            func=mybir.ActivationFunctionType.Exp,
                            scale=SCALE,
                        )
                        wT.append((wt, off))

                    for il in range(NI):
                        ti = NI * g + il
                        ps_o = ps_pv.tile([P, c + 1], f32, name="psmo", tag="pv")
                        for jb in range(ti + 1):
                            wt, off = wT[jb]
                            lo = il * P - off
                            nc.tensor.matmul(
                                ps_o,
                                mm(wt[:, lo : lo + P]),
                                mm(v_t[jb][:]),
                                start=(jb == 0),
                                stop=(jb == ti),
                            )
                        recip = out_pool.tile([P, 1], f32, name="recip", tag="recip")
                        nc.vector.reciprocal(recip, ps_o[:, c : c + 1])
                        ob = out_pool.tile([P, c], f32, name="ob", tag="ob")
                        nc.vector.tensor_scalar_mul(ob, ps_o[:, :c], recip)
                        # output on the scalar HW ring (keeps sync ring free
                        # for next-batch input prefetch)
                        nc.scalar.dma_start(
                            out=y_d[b, ti * P : (ti + 1) * P, :], in_=ob
                        )

    nc.compile()
    return nc


def _get_nc(bpc, t, c):
    key = (bpc, t, c, CDT_NAME)
    if key not in _cache:
        _cache[key] = _build(bpc, t, c)
    return _cache[key]


def run(x, Wq, Wk, Wv, trace=False):
    """Run on hardware; returns (y, BassKernelResults)."""
    from concourse.bass_utils import run_bass_kernel_spmd

    x = np.ascontiguousarray(np.asarray(x, dtype=np.float32))
    Wq = np.asarray(Wq, dtype=np.float32)
    Wk = np.asarray(Wk, dtype=np.float32)
    Wv = np.ascontiguousarray(np.asarray(Wv, dtype=np.float32))
    b, t, c = x.shape
    assert b % N_CORES == 0
    bpc = b // N_CORES

    # Host weight prep: TT = Wk Wq^T with columns in sigma order
    # (position ca*128+qa holds a = 3*qa+ca, matching the device layout).
    tt = (Wk.astype(np.float64) @ Wq.astype(np.float64).T).astype(np.float32)
    perm = np.concatenate([3 * np.arange(P) + ca for ca in range(NCC)])
    tt = np.ascontiguousarray(tt[:, perm])

    nc = _get_nc(bpc, t, c)
    core_ids = list(range(N_CORES))
    in_maps = [
        {"x": x[i * bpc : (i + 1) * bpc], "tt": tt, "wv": Wv}
        for i in core_ids
    ]
    res = run_bass_kernel_spmd(nc, in_maps, core_ids, trace=trace)
    y = np.concatenate([res.results[i]["y"] for i in core_ids], axis=0)
    return y, res


def kernel(x, Wq, Wk, Wv):
    y, _ = run(x, Wq, Wk, Wv, trace=False)
    return y


# revision 16
# speedup vs baseline: 1.0187x; 1.0121x over previous
"""Causal single-head attention (B=16, T=2048, C=HEAD=384) on 8 trn2 cores.

Sharding: data-parallel over batch. Each core gets 2 batch elements and
runs the identical Bass program; results are concatenated on the host.

Math trick: scores = q @ k^T = x @ (Wq Wk^T) @ x^T. The host precomputes
TT = Wk Wq^T (weight-only prep), so per batch the kernel computes a
single projection kAT = TT^T-contracted projection of x^T (instead of
both q and k); the scores matmul streams x^T directly:
    scoresT[s, t] = sum_a kAT[a, s] * xT[a, t].

DMA-descriptor layouts (the DMA rings are descriptor-rate bound, so every
HBM-side run must be long and contiguous):
  * x is loaded interleaved: t = 512a + 4p + n (a = query group, p =
    partition, n in [0,4)), one 6KB contiguous run per (partition, a).
    The PE transposes (needed anyway for x^T) absorb the layout; the
    psum->xT copies write contiguous and read psum strided (cheap).
  * TT/Wv are loaded with the contraction axis permuted: partition q
    holds rows c = 3q+j (4.6KB runs). A contraction axis only needs
    consistent ordering on both matmul operands, so x's c-axis is
    deinterleaved to the same sigma order during the bf16 cast, and TT's
    column axis is sigma-permuted on the host.
  * Output DMAs issue from the scalar engine's HW ring so their waits
    don't block next-batch input DMAs on the sync ring.

Per-core program (per batch element):
  1. DMA x in 4 group-chunks, cast bf16 (deinterleaving c), PE-transpose
     into per-(sigma-chunk, group) xT tiles [128, 512].
  2. kAT = TT-contraction @ x^T; v = x @ Wv ([T, HEAD+1], last col = 1).
  3. Per 512-wide query group g, per causal key block jb: scoresT in
     PSUM fp32 (diagonal blocks narrowed + additive causal mask), evict
     with ACT exp(scale * .) -> bf16.
  4. PV: out = sum_jb weiT^T @ v_ext; the ones column gives the softmax
     denominator in out[:, C]; multiply by its reciprocal, DMA out.

No max-subtraction in softmax: scores*scale are ~N(0,1) for these inputs
so exp cannot overflow fp32; mathematically identical to the reference.
"""

import os
import sys

import numpy as np

for _p in ("/opt/trn_rl_repo",):
    if os.path.isdir(_p) and _p not in sys.path:
        sys.path.append(_p)

B, T, C = 16, 2048, 384
N_CORES = 8
BPC = B // N_CORES  # batch elements per core
P = 128
NCC = C // P  # 3 contraction chunks over C (and over HEAD, since HEAD == C)
GW = 512  # query-group width
NI = GW // P  # 4: interleave factor (inner rows per partition per group)
SCALE = float(C) ** -0.5
MASK_BIG = -1e9

# Compute dtype for matmul operands: "bf16" (fastest), "f32r", or "f32".
CDT_NAME = os.environ.get("ATTN_CDT", "bf16")

_cache = {}


def _build(bpc, t, c):
    import concourse.bass as bass  # noqa: F401
    import concourse.mybir as mybir
    from concourse import bacc
    from concourse.masks import make_identity
    from concourse.tile import TileContext

    f32 = mybir.dt.float32
    nt = t // P  # 16 t-blocks
    ng = t // GW  # 4 query groups (= x DMA chunks per batch)

    if CDT_NAME == "bf16":
        cdt = mybir.dt.bfloat16
        mm_cast = None
    elif CDT_NAME == "f32r":
        cdt = f32
        mm_cast = mybir.dt.float32r
    else:
        cdt = f32
        mm_cast = None

    def mm(ap):
        return ap.bitcast(mm_cast) if mm_cast is not None else ap

    nc = bacc.Bacc("TRN2", target_bir_lowering=False)

    x_d = nc.declare_dram_parameter("x", [bpc, t, c], f32, isOutput=False)
    tt_d = nc.declare_dram_parameter("tt", [c, c], f32, isOutput=False)
    wv_d = nc.declare_dram_parameter("wv", [c, c], f32, isOutput=False)
    y_d = nc.declare_dram_parameter("y", [bpc, t, c], f32, isOutput=True)

    small = cdt != f32
    cast_x = cdt != f32

    with TileContext(nc) as tc:
        with (
            tc.tile_pool(name="singles", bufs=1) as singles,
            tc.tile_pool(name="wstage", bufs=1) as wstage,
            tc.tile_pool(name="xf", bufs=3) as xf_pool,
            tc.tile_pool(name="xb", bufs=3) as xb_pool,
            tc.tile_pool(name="xT", bufs=2 if small else 1) as xT_pool,
            tc.tile_pool(name="kAT", bufs=2 if small else 1) as kAT_pool,
            tc.tile_pool(name="v", bufs=nt + 4 if small else nt + 1) as v_pool,
            tc.tile_pool(name="wT", bufs=nt + 4 if small else nt + 1) as wT_pool,
            tc.tile_pool(name="outp", bufs=4) as out_pool,
            tc.tile_pool(name="ps_t", bufs=2, space="PSUM") as ps_t,
            tc.tile_pool(name="ps_kv", bufs=2, space="PSUM") as ps_kv,
            tc.tile_pool(name="ps_sc", bufs=2, space="PSUM") as ps_sc,
            tc.tile_pool(name="ps_pv", bufs=2, space="PSUM") as ps_pv,
        ):
            ident = singles.tile([P, P], cdt)
            make_identity(nc, ident)

            # mask[s, 3P + u] = 0 if u >= s else MASK_BIG; the slice
            # mask[:, 3P : 3P + N] masks every (narrowed) diagonal block.
            mw = GW + 3 * P
            mask = singles.tile([P, mw], f32)
            nc.gpsimd.memset(mask, 0.0)
            nc.gpsimd.affine_select(
                out=mask,
                in_=mask,
                compare_op=mybir.AluOpType.is_ge,
                fill=MASK_BIG,
                base=-3 * P,
                pattern=[[1, mw]],
                channel_multiplier=-1,
            )

            # ---- batch-0 x DMAs first so the sync ring starts on them ----
            xT_all = []  # per batch: xT[j][a] tiles
            xq_all = []  # per batch: list of (xf, xb) per chunk
            for b in range(bpc):
                xT_all.append(
                    [
                        [
                            xT_pool.tile(
                                [P, GW], cdt, name=f"xT{j}_{a}", tag=f"xT{j}_{a}"
                            )
                            for a in range(ng)
                        ]
                        for j in range(NCC)
                    ]
                )

            def load_x_chunk(b, a):
                # chunk a: rows t = 512a + 4p + n; per-partition 6KB run
                xv = x_d[b].rearrange("(a p n) c -> p a n c", a=ng, n=NI)
                xf = xf_pool.tile([P, NI, c], f32, name="xf", tag="xf")
                nc.sync.dma_start(out=xf, in_=xv[:, a, :, :])
                if cast_x:
                    # deinterleave c to sigma order during the cast:
                    # xb[:, n, j, qq] = x[.., c=3qq+j]
                    xb = xb_pool.tile([P, NI, NCC, P], cdt, name="xb", tag="xb")
                    nc.vector.tensor_copy(
                        xb, xf.rearrange("p n (qq j) -> p n j qq", j=NCC)
                    )
                    return xb
                return xf.rearrange("p n (qq j) -> p n j qq", j=NCC)

            def transpose_x_chunk(b, a, xb):
                for j in range(NCC):
                    pst = ps_t.tile([P, NI, P], cdt, name="pst", tag="pst")
                    for n in range(NI):
                        nc.tensor.transpose(pst[:, n, :], xb[:, n, j, :], ident)
                    # dest position 4*p + n == t_local; contiguous writes,
                    # strided psum reads
                    nc.vector.tensor_copy(
                        xT_all[b][j][a].rearrange("q (p4 n4) -> q p4 n4", n4=NI),
                        pst.rearrange("q n p -> q p n"),
                    )

            # batch-0 chunk 0 first on the sync ring, split into two
            # half-chunks so PE starts as early as possible
            xv0 = x_d[0].rearrange("(a p n) c -> p a n c", a=ng, n=NI)
            for h in range(2):
                xfh = xf_pool.tile([P, NI // 2, c], f32, name="xfh", tag="xf")
                nc.sync.dma_start(
                    out=xfh, in_=xv0[:, 0, h * (NI // 2) : (h + 1) * (NI // 2), :]
                )
                if cast_x:
                    xbh = xb_pool.tile(
                        [P, NI // 2, NCC, P], cdt, name="xbh", tag="xb"
                    )
                    nc.vector.tensor_copy(
                        xbh, xfh.rearrange("p n (qq j) -> p n j qq", j=NCC)
                    )
                else:
                    xbh = xfh.rearrange("p n (qq j) -> p n j qq", j=NCC)
                for j in range(NCC):
                    psh = ps_t.tile([P, NI // 2, P], cdt, name="psh", tag="pst")
                    for n in range(NI // 2):
                        nc.tensor.transpose(psh[:, n, :], xbh[:, n, j, :], ident)
                    # dest t_local = 4*p + (2h + n): stride-4 pairs
                    nc.vector.tensor_copy(
                        xT_all[0][j][0].rearrange("q (p4 n4) -> q p4 n4", n4=NI)[
                            :, :, h * (NI // 2) : (h + 1) * (NI // 2)
                        ],
                        psh.rearrange("q n p -> q p n"),
                    )

            # ---- weights (rows c = 3q+j per partition; 4.6KB runs) ----
            w_sb = []
            for name, wd in (("tt", tt_d), ("wv", wv_d)):
                stage = wstage.tile(
                    [P, NCC, c], f32, name=f"wst_{name}", tag=f"wst_{name}"
                )
                nc.sync.dma_start(
                    out=stage, in_=wd[:].rearrange("(q j) h -> q j h", j=NCC)
                )
                if cdt == f32:
                    w_sb.append(stage)
                else:
                    wb = singles.tile(
                        [P, NCC, c], cdt, name=f"{name}b", tag=f"{name}b"
                    )
                    nc.vector.tensor_copy(wb, stage)
                    w_sb.append(wb)
            TT, WV = w_sb

            for b in range(bpc):
                xT = xT_all[b]
                kAT = [
                    [
                        kAT_pool.tile(
                            [P, GW], cdt, name=f"kAT{ca}_{a}", tag=f"kAT{ca}_{a}"
                        )
                        for a in range(ng)
                    ]
                    for ca in range(NCC)
                ]
                v_t = []

                # chunk-major: transpose(a) -> kAT(a) -> v(a) -> attention(a)
                # keeps PE dense from the first chunk onward.
                for g in range(ng):
                    if b > 0 or g > 0:
                        xbg = load_x_chunk(b, g)
                        transpose_x_chunk(b, g, xbg)

                    # kAT for this chunk
                    for ca in range(NCC):
                        ps = ps_kv.tile([P, GW], f32, name="pskv", tag="kv")
                        for cc in range(NCC):
                            nc.tensor.matmul(
                                ps,
                                mm(TT[:, cc, ca * P : (ca + 1) * P]),
                                mm(xT[cc][g]),
                                start=(cc == 0),
                                stop=(cc == NCC - 1),
                            )
                        nc.vector.tensor_copy(kAT[ca][g], ps)

                    # v for this chunk's 4 t-blocks
                    for n in range(NI * g, NI * g + NI):
                        vt = v_pool.tile([P, c + 1], cdt, name="vt", tag="v")
                        ps = ps_kv.tile([P, GW], f32, name="pskv", tag="kv")
                        for cc in range(NCC):
                            nc.tensor.matmul(
                                ps[:, :c],
                                mm(
                                    xT[cc][n // NI][
                                        :, (n % NI) * P : (n % NI + 1) * P
                                    ]
                                ),
                                mm(WV[:, cc, :]),
                                start=(cc == 0),
                                stop=(cc == NCC - 1),
                            )
                        nc.vector.tensor_copy(vt[:, :c], ps[:, :c])
                        nc.vector.memset(vt[:, c : c + 1], 1.0)
                        v_t.append(vt)

                    # ---- attention for query group g ----
                    nblk = NI * g + NI  # causal: s-blocks 0 .. 4g+3
                    wT = []  # (tile, first-valid t_local) per jb
                    for jb in range(nblk):
                        dv = jb - NI * g  # >= 0: diagonal block, narrowed
                        off = max(dv, 0) * P
                        n_free = GW - off
                        ps = ps_sc.tile([P, GW], f32, name="pssc", tag="sc")
                        for cc in range(NCC):
                            nc.tensor.matmul(
                                ps[:, :n_free],
                                mm(
                                    kAT[cc][jb // NI][
                                        :, (jb % NI) * P : (jb % NI + 1) * P
                                    ]
                                ),
                                mm(xT[cc][g][:, off:]),
                                start=(cc == 0),
                                stop=(cc == NCC - 1),
                            )
                        if dv >= 0:
                            nc.vector.tensor_add(
                                ps[:, :n_free],
                                ps[:, :n_free],
                                mask[:, 3 * P : 3 * P + n_free],
                            )
                        wt = wT_pool.tile([P, GW], cdt, name="wTt", tag="wT")
                        nc.scalar.activation(
                            out=wt[:, :n_free],
                            in_=ps[:, :n_free],
                            func=mybir.ActivationFunctionType.Exp,
                            scale=SCALE,
                        )
                        wT.append((wt, off))

                    for il in range(NI):
                        ti = NI * g + il
                        ps_o = ps_pv.tile([P, c + 1], f32, name="psmo", tag="pv")
                        for jb in range(ti + 1):
                            wt, off = wT[jb]
                            lo = il * P - off
                            nc.tensor.matmul(
                                ps_o,
                                mm(wt[:, lo : lo + P]),
                                mm(v_t[jb][:]),
                                start=(jb == 0),
                                stop=(jb == ti),
                            )
                        recip = out_pool.tile([P, 1], f32, name="recip", tag="recip")
                        nc.vector.reciprocal(recip, ps_o[:, c : c + 1])
                        ob = out_pool.tile([P, c], f32, name="ob", tag="ob")
                        nc.vector.tensor_scalar_mul(ob, ps_o[:, :c], recip)
                        # output on the scalar HW ring (keeps sync ring free
                        # for next-batch input prefetch)
                        nc.scalar.dma_start(
                            out=y_d[b, ti * P : (ti + 1) * P, :], in_=ob
                        )

    nc.compile()
    return nc


def _get_nc(bpc, t, c):
    key = (bpc, t, c, CDT_NAME)
    if key not in _cache:
        _cache[key] = _build(bpc, t, c)
    return _cache[key]


def run(x, Wq, Wk, Wv, trace=False):
    """Run on hardware; returns (y, BassKernelResults)."""
    from concourse.bass_utils import run_bass_kernel_spmd

    x = np.ascontiguousarray(np.asarray(x, dtype=np.float32))
    Wq = np.asarray(Wq, dtype=np.float32)
    Wk = np.asarray(Wk, dtype=np.float32)
    Wv = np.ascontiguousarray(np.asarray(Wv, dtype=np.float32))
    b, t, c = x.shape
    assert b % N_CORES == 0
    bpc = b // N_CORES

    # Host weight prep: TT = Wk Wq^T with columns in sigma order
    # (position ca*128+qa holds a = 3*qa+ca, matching the device layout).
    tt = (Wk.astype(np.float64) @ Wq.astype(np.float64).T).astype(np.float32)
    perm = np.concatenate([3 * np.arange(P) + ca for ca in range(NCC)])
    tt = np.ascontiguousarray(tt[:, perm])

    nc = _get_nc(bpc, t, c)
    core_ids = list(range(N_CORES))
    in_maps = [
        {"x": x[i * bpc : (i + 1) * bpc], "tt": tt, "wv": Wv}
        for i in core_ids
    ]
    res = run_bass_kernel_spmd(nc, in_maps, core_ids, trace=trace)
    y = np.concatenate([res.results[i]["y"] for i in core_ids], axis=0)
    return y, res


def kernel(x, Wq, Wk, Wv):
    y, _ = run(x, Wq, Wk, Wv, trace=False)
    return y
